# revision 1
# baseline (speedup 1.0000x reference)
"""Channel-attention (single-head shared attention over channels) Trainium2 kernel.

Reference computation (per batch b, C=512 channels, N=64*64=4096 spatial):
    xf = x[b].reshape(C, N)
    q = wq[:,None]*xf ; k = wk[:,None]*xf ; v = wv[:,None]*xf
    attn = softmax(q @ k.T / sqrt(N), axis=-1)        # (C, C)
    out[b] = (attn @ v).reshape(C, H, W)

Kernel strategy (data-parallel over B across 8 cores, 2 batches/core):
  G = xf @ xf.T is computed once (symmetric); the row/col scales wq, wk are
  folded afterwards.  We directly produce the TRANSPOSED logits
  S[d,c] = wk[d]*wq[c]*G[d,c]/sqrt(N)  (== attn_pre[c,d]); exp() of that is
  exactly the lhsT the second matmul needs, so no transpose of the attention
  matrix is ever required.  Softmax normalization (1/Z) is deferred past the
  second matmul (linearity) and applied as a per-partition scale on the
  output tiles.  Z[c] = sum_d E[d,c] is obtained on the tensor engine by
  multiplying the scaled-E lhsT with a matching reciprocal column vector.
  The wq/sqrt(N) factor is folded into the x->bf16 cast (per-channel, so the
  Gram PSUM output is exp()-ready with only a per-partition ACT scale);
  the resulting scale on the second matmul's rhs is compensated exactly in
  the E scaling (sqrt(N)*wv/wq) computed on the host in float64.

  x is cast to bf16 once; the [N,C] transposed copy needed for the Gram
  matmul is produced with tensor-engine transposes (the PE is idle during
  the input phase; XBAR DMA transposes would globally serialize the DMA
  engines at every copy<->transpose mode switch).  G is symmetric, so only
  the upper-triangular block-columns are computed (rhs width shrinks
  512/384/256/128 per d-chunk) and the lower blocks are reconstructed by
  transposing the computed ones before the (asymmetric) scale+exp.
"""

import numpy as np
import ml_dtypes

import concourse.bass as bass
import concourse.tile as tile
from concourse import mybir
from concourse.bass_utils import run_bass_kernel_spmd
from concourse.masks import make_identity

P = 128
C = 512
N = 4096
B_TOTAL = 16
N_CORES = 8
B_PER_CORE = B_TOTAL // N_CORES
CI = C // P        # 4 channel chunks
NT = N // P        # 32 spatial tiles of 128
NCH = 8            # n is staged/cast in chunks of 512
NW = N // 512      # 8 output column tiles of 512
F32 = mybir.dt.float32
BF16 = mybir.dt.bfloat16


def _split_multiwaits(nc):
    """Workaround: this walrus build rejects instructions carrying >1 sync
    wait ("Too many sync wait commands").  Hoist all but the last wait onto
    standalone EventSemaphore instructions placed just before the owner (same
    engine, so sequencer order preserves semantics)."""
    for f in nc.m.functions:
        for blk in f.blocks:
            new_insts = []
            for ins in blk.instructions:
                si = ins.sync_info
                if si is not None and si.on_wait is not None and len(si.on_wait) > 1:
                    waits = list(si.on_wait)
                    for k, w in enumerate(waits[:-1]):
                        new_insts.append(
                            mybir.InstEventSemaphore(
                                name=f"{ins.name}_splitw{k}",
                                engine=ins.engine,
                                sync_info=mybir.SyncInfo(on_wait=[w], on_update=[]),
                            )
                        )
                    si.on_wait = [waits[-1]]
                new_insts.append(ins)
            blk.instructions[:] = new_insts


def build_kernel():
    nc = bass.Bass()
    x_in = nc.dram_tensor("x", [B_PER_CORE, C, N], F32, kind="ExternalInput")
    # packed f32 weights, all in column layout w[p, i] = w[i*128 + p]:
    # uc = wq/sqrt(N) (folded into the x cast), fc = sqrt(N)*wk/wq (exp
    # scale), wvc2 = sqrt(N)*wv/wq (evw scale)
    wpack_in = nc.dram_tensor("wpack", [P, 3 * CI], F32, kind="ExternalInput")
    winv_in = nc.dram_tensor("winv", [P, CI], BF16, kind="ExternalInput")
    out = nc.dram_tensor("out", [B_PER_CORE, C, N], F32, kind="ExternalOutput")

    with tile.TileContext(nc) as tc:
        with (
            tc.tile_pool(name="singles", bufs=1) as singles,
            tc.tile_pool(name="stage", bufs=6) as stage,
            tc.tile_pool(name="xbf", bufs=2) as xbf_pool,
            tc.tile_pool(name="xt", bufs=1) as xt_pool,
            tc.tile_pool(name="sm", bufs=2) as sm_pool,
            tc.tile_pool(name="evw", bufs=2) as evw_pool,
            tc.tile_pool(name="osb", bufs=3) as osb_pool,
            tc.tile_pool(name="rz", bufs=8) as rz_pool,
            tc.tile_pool(name="gp", bufs=4, space="PSUM") as gp_pool,
            tc.tile_pool(name="op", bufs=4, space="PSUM") as op_pool,
        ):
            wpack = singles.tile([P, 3 * CI], F32)
            winv = singles.tile([P, CI], BF16)
            uc = wpack[:, 0:CI]
            fc = wpack[:, CI : 2 * CI]
            wvc2 = wpack[:, 2 * CI : 3 * CI]
            # wpack is tiny now (12 f32 cols) and the very first cast needs
            # uc -- load weights up front
            nc.sync.dma_start(wpack, wpack_in[:, :])
            nc.sync.dma_start(winv, winv_in[:, :])
            ident = singles.tile([P, P], BF16)
            make_identity(nc, ident)
            identf = singles.tile([P, P], F32)
            make_identity(nc, identf)

            for b in range(B_PER_CORE):
                xr = x_in[b].rearrange("(i p) n -> p i n", p=P)  # [128, 4, 4096]

                # ---- load + cast to bf16, and transpose (pipelined per chunk).
                # Transposes go through the tensor engine (PE is idle during
                # the input phase anyway): DMA(XBAR) transposes would force
                # global copy<->transpose serialization of the DMA engines.
                xbf = xbf_pool.tile([P, CI, N], BF16)     # [128, 4, 4096]
                xt = xt_pool.tile([P, NT, C], BF16)       # [128, 32, 512]
                # first chunk of the first batch is split in two so the
                # first transposes (and matmuls) start ~1.5us earlier
                if b == 0:
                    chunks = [(0, 128), (128, 128), (256, 256)] + [
                        (ns * 512, 512) for ns in range(1, NCH)
                    ]
                else:
                    chunks = [(ns * 512, 512) for ns in range(NCH)]
                for ichunk, (n0, nw) in enumerate(chunks):
                    nsl = slice(n0, n0 + nw)
                    stg = stage.tile([P, CI, 512], F32, tag="stg")
                    nc.sync.dma_start(stg[:, :, :nw], xr[:, :, nsl])
                    # y = (wq/sqrt(N)) * x, folded into the bf16 cast;
                    # split per channel-chunk across DVE and ACT
                    for ci in range(CI):
                        if ci % 2 == 0:
                            nc.vector.tensor_scalar_mul(
                                xbf[:, ci, nsl],
                                stg[:, ci, :nw],
                                uc[:, ci : ci + 1],
                            )
                        else:
                            nc.scalar.activation(
                                xbf[:, ci, nsl],
                                stg[:, ci, :nw],
                                func=mybir.ActivationFunctionType.Copy,
                                scale=uc[:, ci : ci + 1],
                            )
                    for jj in range(nw // P):
                        j = n0 // P + jj
                        jsl = slice(n0 + jj * P, n0 + (jj + 1) * P)
                        for ci in range(CI):
                            tp = op_pool.tile(
                                [P, 512], BF16, tag="op", name=f"tp_{b}_{j}_{ci}"
                            )
                            nc.tensor.transpose(tp[:, :P], xbf[:, ci, jsl], ident)
                            # psum -> SBUF (cast to bf16); alternate DVE/ACT
                            dst = xt[:, j, ci * P : (ci + 1) * P]
                            if (jj * CI + ci) % 8 < 6:
                                nc.vector.tensor_copy(out=dst, in_=tp[:, :P])
                            else:
                                nc.scalar.activation(
                                    dst,
                                    tp[:, :P],
                                    func=mybir.ActivationFunctionType.Copy,
                                )

                # ---- Gram matmul (j-outer: consume xt as it is produced) ----
                gps = [
                    gp_pool.tile([P, C], F32, tag="gp", name=f"gp{dc}_{b}")
                    for dc in range(CI)
                ]
                # G is symmetric: compute only block-columns >= dc for each
                # d-chunk (rhs width shrinks 512/384/256/128); the lower
                # blocks are reconstructed by transposing the upper ones.
                # Last 4 j's run dc-major so gp[0] finishes ~2us before
                # gp[3]: its exp chain overlaps the mm1 tail.
                for j in range(NT - 8):
                    for dc in range(CI):
                        nc.tensor.matmul(
                            gps[dc][:, dc * P :],
                            lhsT=xt[:, j, dc * P : (dc + 1) * P],
                            rhs=xt[:, j, dc * P :],
                            start=(j == 0),
                            stop=False,
                        )
                for dc in range(CI):
                    for j in range(NT - 8, NT):
                        nc.tensor.matmul(
                            gps[dc][:, dc * P :],
                            lhsT=xt[:, j, dc * P : (dc + 1) * P],
                            rhs=xt[:, j, dc * P :],
                            start=False,
                            stop=(j == NT - 1),
                        )
                # ---- scale + exp (upper blocks straight from PSUM) ----
                evw = evw_pool.tile([P, CI, C], BF16)     # exp(S)*wv, bf16
                for dc in range(CI):
                    # gp already holds u_d*u_c*G; exp(fc_d * gp) = E^T, then
                    # evw = E^T * (sqrt(N)*wv/wq)_d compensates the u-scaled
                    # mm2 rhs exactly
                    w = (CI - dc) * P
                    e = sm_pool.tile([P, C], F32, tag="e")
                    nc.scalar.activation(
                        e[:, :w],
                        gps[dc][:, dc * P :],
                        func=mybir.ActivationFunctionType.Exp,
                        scale=fc[:, dc : dc + 1],
                    )
                    nc.vector.tensor_scalar_mul(
                        evw[:, dc, dc * P :], e[:, :w], wvc2[:, dc : dc + 1]
                    )
                # ---- mirror blocks: evw[dc][:, ci<dc] from G[ci][dc]^T ----
                for ci in range(CI):
                    for dc in range(ci + 1, CI):
                        gsb = sm_pool.tile([P, P], F32, tag="gsb")
                        nc.vector.tensor_copy(
                            out=gsb, in_=gps[ci][:, dc * P : (dc + 1) * P]
                        )
                        mt = op_pool.tile(
                            [P, P], F32, tag="op", name=f"mt_{b}_{ci}_{dc}"
                        )
                        nc.tensor.transpose(mt, gsb, identf)
                        me = sm_pool.tile([P, P], F32, tag="me")
                        nc.scalar.activation(
                            me,
                            mt,
                            func=mybir.ActivationFunctionType.Exp,
                            scale=fc[:, dc : dc + 1],
                        )
                        nc.vector.tensor_scalar_mul(
                            evw[:, dc, ci * P : (ci + 1) * P],
                            me,
                            wvc2[:, dc : dc + 1],
                        )

                # ---- second matmul + deferred softmax normalization ----
                for cc in range(CI - 1, -1, -1):
                    csl = slice(cc * P, (cc + 1) * P)
                    zpt = gp_pool.tile([P, C], F32, tag="gp", name=f"zp_{b}_{cc}")
                    zp = zpt[:, 0:1]
                    for dc in range(CI):
                        nc.tensor.matmul(
                            zp,
                            lhsT=evw[:, dc, csl],
                            rhs=winv[:, dc : dc + 1],
                            start=(dc == 0),
                            stop=(dc == CI - 1),
                        )
                    rz = rz_pool.tile([P, 1], F32)
                    nc.vector.reciprocal(rz, zp)
                    for h in range(2):
                        osb = osb_pool.tile([P, 4, 512], F32)
                        for q in range(4):
                            nt = h * 4 + q
                            ntl = slice(nt * 512, (nt + 1) * 512)
                            op = op_pool.tile([P, 512], F32)
                            for dc in range(CI):
                                nc.tensor.matmul(
                                    op,
                                    lhsT=evw[:, dc, csl],
                                    rhs=xbf[:, dc, ntl],
                                    start=(dc == 0),
                                    stop=(dc == CI - 1),
                                )
                            # deferred softmax 1/Z on the (otherwise idle)
                            # scalar engine: osb = Copy(op * rz)
                            nc.scalar.activation(
                                osb[:, q, :],
                                op,
                                func=mybir.ActivationFunctionType.Copy,
                                scale=rz,
                            )
                            if b == B_PER_CORE - 1 and cc <= 1:
                                # tail: store per quarter so the final DMA
                                # overlaps the last compute
                                nc.sync.dma_start(out[b, csl, ntl], osb[:, q, :])
                        if not (b == B_PER_CORE - 1 and cc <= 1):
                            nc.sync.dma_start(
                                out[b, csl, h * 2048 : (h + 1) * 2048], osb
                            )

    _split_multiwaits(nc)
    return nc


_NC_CACHE = None


def _get_nc():
    global _NC_CACHE
    if _NC_CACHE is None:
        _NC_CACHE = build_kernel()
    return _NC_CACHE


def make_weight_inputs(wq, wk, wv):
    wq = np.asarray(wq, np.float64)
    wk = np.asarray(wk, np.float64)
    wv = np.asarray(wv, np.float64)
    # guard against exact zeros in wq (divisor)
    wqg = np.where(np.abs(wq) < 1e-30, 1e-30, wq)
    rn = np.sqrt(np.float64(N))
    u = wqg / rn              # folded into the x->bf16 cast
    f = rn * wk / wqg         # exp scale: f * (u_d u_c G) = wk_d wq_c G / rn
    wv2 = rn * wv / wqg       # evw scale: E * wv2 compensates u-scaled rhs
    uc = u.reshape(CI, P).T
    fcl = f.reshape(CI, P).T
    wvc2 = wv2.reshape(CI, P).T
    wpack = np.concatenate([uc, fcl, wvc2], axis=1).astype(np.float32)
    wv2_bf = wv2.astype(np.float32).astype(ml_dtypes.bfloat16)
    winv = (1.0 / wv2_bf.astype(np.float32)).astype(ml_dtypes.bfloat16)
    winv = winv.reshape(CI, P).T.copy()
    return wpack, winv


def kernel(x: np.ndarray, wq: np.ndarray, wk: np.ndarray, wv: np.ndarray) -> np.ndarray:
    assert x.shape == (B_TOTAL, C, 64, 64) and x.dtype == np.float32
    nc = _get_nc()

    wpack, winv = make_weight_inputs(wq, wk, wv)
    xr = np.ascontiguousarray(x.reshape(B_TOTAL, C, N))
    in_maps = []
    for core in range(N_CORES):
        in_maps.append(
            {
                "x": xr[core * B_PER_CORE : (core + 1) * B_PER_CORE],
                "wpack": wpack,
                "winv": winv,
            }
        )

    res = run_bass_kernel_spmd(nc, in_maps, core_ids=list(range(N_CORES)))
    outs = [r["out"] for r in res.results]
    return np.concatenate(outs, axis=0).reshape(B_TOTAL, C, 64, 64)

